# revision 1
# baseline (speedup 1.0000x reference)
"""Multi-head self-attention (B=4, S=2048, D=1024, H=16) on 8 TRN2 NeuronCores.

Sharding: batch x head-group. Core c handles batch b=c//2 and heads
[8*(c%2), 8*(c%2)+8). Each core computes QKV projection, attention and its
partial output projection; the host sums the two head-group partials per batch
and adds b_proj.

Per-core dataflow (all matmuls float32r = full PE rate, ~1.5e-4 rounding):
  stage 1: Y^T = [Q^T; K^T] = wqk^T-free matmul(lhsT=wqk, rhs=x^T) -> [1024f, 2048t]
           V   = matmul(lhsT=x^T chunk, rhs=wv)                    -> [2048t, 512f]
  stage 2: S^T[k,q] = K_h Q_h^T via row-tiled head pairs (d=64 contraction)
           P^T = exp(S^T * 0.125) on ACT (ScalarE), reading 2-bank PSUM tiles
  stage 3: C~^T = [V_h|1]^T P^T  (ones-column makes row 64 the softmax denom)
           normalize: recip(sums) -> DRAM -> partition-broadcast DMA -> DVE mul
  stage 4: out = C^T-proj: matmul(lhsT=C^T chunk, rhs=w_proj rows)  -> [2048t, 1024]
"""
import numpy as np

import concourse.bacc as bacc
import concourse.tile as tile
from concourse import bass_isa, mybir
from concourse import bass_utils

P = 128
B, S, D = 4, 2048, 1024
H_TOT, HD = 16, 64
H = 8          # heads per core
NPAIR = 4      # head pairs per core
SCALE = HD ** -0.5
DCH = D // P   # 8 contraction chunks
NTT = S // P   # 16 token tiles
f32 = mybir.dt.float32
f32r = mybir.dt.float32r
AF = mybir.ActivationFunctionType

_CACHED_NC = None


def build_nc():
    nc = bacc.Bacc(trn_type="TRN2", target_bir_lowering=False, debug=False)
    xt = nc.dram_tensor("xt", [D, S], f32r, kind="ExternalInput").ap()
    wqk = nc.dram_tensor("wqk", [D, 2 * H * HD], f32r, kind="ExternalInput").ap()
    wv = nc.dram_tensor("wv", [D, H * HD], f32r, kind="ExternalInput").ap()
    wp = nc.dram_tensor("wp", [H * HD, D], f32r, kind="ExternalInput").ap()
    bqk = nc.dram_tensor("bqk", [8, P], f32, kind="ExternalInput").ap()
    vbias = nc.dram_tensor("vbias", [P, NPAIR * 130], f32, kind="ExternalInput").ap()
    out = nc.dram_tensor("out", [S, D], f32, kind="ExternalOutput").ap()

    with tile.TileContext(nc) as tc:
        with tc.tile_pool(name="persist", bufs=1) as pp:
            # persistent SBUF tensors
            yt = [pp.tile([P, S], f32r, name=f"yt{f}") for f in range(8)]
            vp = pp.tile([P, NTT, NPAIR * 130], f32r, name="vp")
            vbias_t = pp.tile([P, NPAIR * 130], f32, name="vbias_t")

            # ---------------- stage 1: QKV projections ----------------
            with (
                tc.tile_pool(name="s1w", bufs=1) as s1w,
                tc.tile_pool(name="s1x", bufs=2) as s1x,
                tc.tile_pool(name="s1ps", bufs=4, space="PSUM") as s1ps,
            ):
                TCH = 256
                NCH = S // TCH
                # per-feature-tile weight tiles so the first matmuls start early
                wqk_f = [s1w.tile([P, DCH, P], f32r, name=f"wqkf{f}")
                         for f in range(8)]
                wv_t = s1w.tile([P, DCH, H * HD], f32r, name="wv_t")
                bqk_t = s1w.tile([P, 8], f32, name="bqk_t")
                wqk4 = wqk.rearrange("(c p) (f g) -> p c f g", p=P, f=8)
                # first xt chunks + weight tiles up front; weights go on the
                # ACT HWDGE ring so they don't queue behind the xt stream
                xt_ts = [s1x.tile([P, DCH, TCH], f32r, name="xt_t")
                         for _ in range(2)]
                nc.sync.dma_start(xt_ts[0][:],
                                  xt[:, 0:TCH].rearrange("(c p) s -> p c s", p=P))
                nc.scalar.dma_start(wqk_f[0][:], wqk4[:, :, 0, :])
                nc.scalar.dma_start(bqk_t[:], bqk.rearrange("a p -> p a"))
                nc.scalar.dma_start(vbias_t[:], vbias[:])
                for f in range(1, 8):
                    nc.scalar.dma_start(wqk_f[f][:], wqk4[:, :, f, :])
                nc.sync.dma_start(
                    xt_ts[1][:],
                    xt[:, TCH:2 * TCH].rearrange("(c p) s -> p c s", p=P))
                nc.scalar.dma_start(wv_t[:], wv.rearrange("(c p) f -> p c f", p=P))

                for t in range(NCH):  # 256-token chunks
                    tsl = slice(t * TCH, (t + 1) * TCH)
                    if t < 2:
                        xt_t = xt_ts[t]
                    else:
                        xt_t = s1x.tile([P, DCH, TCH], f32r, name="xt_t")
                        nc.sync.dma_start(
                            xt_t[:], xt[:, tsl].rearrange("(c p) s -> p c s", p=P))
                    for f in range(8):  # Q,K feature tiles
                        ps = s1ps.tile([P, TCH], f32, name="s1pq")
                        for i in range(DCH):
                            nc.tensor.matmul(
                                ps[:], wqk_f[f][:, i, :], xt_t[:, i, :],
                                start=(i == 0), stop=(i == DCH - 1))
                        nc.vector.tensor_scalar(
                            out=yt[f][:, tsl], in0=ps[:], scalar1=bqk_t[:, f:f + 1],
                            scalar2=None, op0=mybir.AluOpType.add)
                    if t == 0:
                        # vp bias+ones init, deferred so the xt/weight streams
                        # get the startup DMA bandwidth
                        for tt2 in range(NTT):
                            nc.gpsimd.dma_start(vp[:, tt2, :], vbias[:])
                    for sub in range(TCH // P):  # V for 128-token subtiles
                        tt = t * (TCH // P) + sub
                        ps = s1ps.tile([P, 512], f32, name="s1p")
                        for i in range(DCH):
                            nc.tensor.matmul(
                                ps[:], xt_t[:, i, sub * P:(sub + 1) * P], wv_t[:, i, :],
                                start=(i == 0), stop=(i == DCH - 1))
                        vpt = vp[:, tt, :].rearrange("p (j k c) -> p j k c",
                                                     j=NPAIR, k=2)
                        vb4 = vbias_t[:].rearrange("p (j k c) -> p j k c",
                                                   j=NPAIR, k=2)
                        nc.vector.tensor_tensor(
                            out=vpt[:, :, :, 0:HD],
                            in0=ps[:].rearrange("p (j k c) -> p j k c", j=NPAIR, k=2),
                            in1=vb4[:, :, :, 0:HD],
                            op=mybir.AluOpType.add)

            # ---------------- stages 2+3: attention ----------------
            # Per k-chunk: one [128,1024] PSUM tile holds S^T for both heads of
            # the pair (even in cols 0:512, odd in 512:1024), one ACT exp per
            # k-chunk, PV software-pipelined one k-chunk behind.
            ct = [pp.tile([P, S], f32r, name=f"ct{j}") for j in range(NPAIR)]
            with (
                tc.tile_pool(name="s4w", bufs=1) as s4w,
                tc.tile_pool(name="att", bufs=1) as att,
                tc.tile_pool(name="s4o", bufs=2) as s4o,
                tc.tile_pool(name="spt", bufs=2, space="PSUM") as sptp,
                tc.tile_pool(name="cps", bufs=2, space="PSUM") as cpsp,
                tc.tile_pool(name="s4ps", bufs=2, space="PSUM") as s4ps,
            ):
                wp_t = s4w.tile([P, NPAIR, D], f32r, name="wp_t")
                nc.scalar.dma_start(wp_t[:], wp.rearrange("(c p) f -> p c f", p=P))

                # zeros rows 0:63 + per-norm recip row 64; partition all-reduce
                # (add) then replicates the recip row across all partitions
                zt = att.tile([65, 1024], f32, name="zt", bufs=1)
                nc.vector.memset(zt[0:HD, :], 0.0)

                def emit_norm(j, qa, cps_e, cps_o):
                    nc.vector.reciprocal(zt[64:65, 0:512], cps_e[64:65, :])
                    nc.vector.reciprocal(zt[64:65, 512:1024], cps_o[64:65, :])
                    rbc = att.tile([65, 1024], f32, name="rbc", bufs=2)
                    nc.gpsimd.partition_all_reduce(
                        rbc[:], zt[:], channels=65,
                        reduce_op=bass_isa.ReduceOp.add)
                    nc.vector.tensor_mul(ct[j][0:HD, qa], cps_e[0:HD, :],
                                         rbc[0:HD, 0:512])
                    cttmp = att.tile([HD, 512], f32r, name="cttmp", bufs=1)
                    nc.vector.tensor_mul(cttmp[:], cps_o[0:HD, :],
                                         rbc[0:HD, 512:1024])
                    nc.sync.dma_start(ct[j][HD:P, qa], cttmp[:])

                # projection work for one token tile, emitted as a list of
                # closures so matmuls drip into the PE stream without bursts
                def proj_steps(tt):
                    tsl = slice(tt * P, (tt + 1) * P)
                    steps = []
                    state = {}

                    def mk_mm(half, fc):
                        def f():
                            if fc == 0:
                                state[half] = s4ps.tile([P, 512], f32, name="s4p")
                            nc.tensor.matmul(
                                state[half][:], ct[fc][:, tsl],
                                wp_t[:, fc, half * 512:(half + 1) * 512],
                                start=(fc == 0), stop=(fc == NPAIR - 1))
                            if fc == NPAIR - 1:
                                o_sb = s4o.tile([P, 512], f32, name="o_sb",
                                                bufs=4)
                                nc.vector.tensor_copy(o_sb[:], state[half][:])
                                nc.sync.dma_start(
                                    out[tsl, half * 512:(half + 1) * 512],
                                    o_sb[:])
                        return f

                    for half in range(2):
                        for fc in range(NPAIR):
                            steps.append(mk_mm(half, fc))
                    return steps

                norm_pending = None
                proj_queue = []
                for qc in range(4):  # 512-wide query chunks, outer
                    qa = slice(qc * 512, (qc + 1) * 512)
                    for j in range(NPAIR):
                        qt, kt = yt[j], yt[NPAIR + j]
                        cps_e = cps_o = None
                        pv_pending = None
                        for kc in range(NTT):
                            ksl = slice(kc * P, (kc + 1) * P)
                            spt = sptp.tile([P, 1024], f32, name="spt")
                            nc.tensor.matmul(spt[:, 0:512], kt[0:HD, ksl],
                                             qt[0:HD, qa], start=True, stop=True)
                            nc.tensor.matmul(spt[:, 512:1024], kt[HD:P, ksl],
                                             qt[HD:P, qa], start=True, stop=True)
                            ppt = att.tile([P, 1024], f32r, name="ppt", bufs=4)
                            nc.scalar.activation(ppt[:], spt[:], AF.Exp,
                                                 scale=SCALE)
                            if kc == 1 and norm_pending is not None:
                                # previous (qc,j) normalization, deferred past
                                # this iteration's first two S/exp to hide its
                                # recip -> all-reduce -> mul chain
                                emit_norm(*norm_pending)
                                norm_pending = None
                            if pv_pending is not None:
                                if cps_e is None:
                                    cps_e = cpsp.tile([65, 512], f32, name="cps")
                                    cps_o = cpsp.tile([65, 512], f32, name="cps")
                                _emit_pv(nc, cps_e, cps_o, vp, pv_pending[0],
                                         pv_pending[1], j)
                            pv_pending = (kc, ppt)
                            if proj_queue and kc % 2 == 1:
                                proj_queue.pop(0)()  # drip one projection step
                        _emit_pv(nc, cps_e, cps_o, vp, pv_pending[0],
                                 pv_pending[1], j)
                        norm_pending = (j, qa, cps_e, cps_o)
                    # queue projection for this query chunk's 4 token tiles
                    # (runnable once this qc's last norm flushes next sweep)
                    for tt in range(qc * 4, (qc + 1) * 4):
                        proj_queue.extend(proj_steps(tt))
                emit_norm(*norm_pending)
                for step in proj_queue:
                    step()

    nc.finalize()
    return nc


def _emit_pv(nc, cps_e, cps_o, vp, kc, ppt, j):
    nc.tensor.matmul(cps_e[0:65, :], vp[:, kc, j * 130:j * 130 + 65],
                     ppt[:, 0:512], start=(kc == 0), stop=(kc == NTT - 1))
    nc.tensor.matmul(cps_o[0:65, :], vp[:, kc, j * 130 + 65:j * 130 + 130],
                     ppt[:, 512:1024], start=(kc == 0), stop=(kc == NTT - 1))


def get_nc():
    global _CACHED_NC
    if _CACHED_NC is None:
        _CACHED_NC = build_nc()
    return _CACHED_NC


def make_in_maps(x, w_qkv, b_qkv, w_proj):
    """Host-side sharding: one input dict per core."""
    w = np.asarray(w_qkv, np.float32).reshape(D, 3, H_TOT, HD)
    bq3 = np.asarray(b_qkv, np.float32).reshape(3, H_TOT, HD)
    in_maps = []
    for c in range(8):
        b, hg = c // 2, c % 2
        hs = slice(hg * H, (hg + 1) * H)
        wqk_c = np.ascontiguousarray(
            np.concatenate([w[:, 0, hs, :].reshape(D, H * HD),
                            w[:, 1, hs, :].reshape(D, H * HD)], axis=1))
        wv_c = np.ascontiguousarray(w[:, 2, hs, :].reshape(D, H * HD))
        wp_c = np.ascontiguousarray(
            np.asarray(w_proj, np.float32).reshape(H_TOT, HD, D)[hs].reshape(H * HD, D))
        bqk_c = np.ascontiguousarray(
            np.concatenate([bq3[0, hs].reshape(H * HD),
                            bq3[1, hs].reshape(H * HD)]).reshape(8, P))
        bv = bq3[2, hs].reshape(H * HD)
        vbias_c = np.zeros((P, NPAIR * 130), np.float32)
        for j in range(NPAIR):
            vbias_c[:, j * 130:j * 130 + HD] = bv[(2 * j) * HD:(2 * j + 1) * HD]
            vbias_c[:, j * 130 + HD] = 1.0
            vbias_c[:, j * 130 + 65:j * 130 + 65 + HD] = \
                bv[(2 * j + 1) * HD:(2 * j + 2) * HD]
            vbias_c[:, j * 130 + 129] = 1.0
        xt_c = np.ascontiguousarray(np.asarray(x[b], np.float32).T)
        in_maps.append({"xt": xt_c, "wqk": wqk_c, "wv": wv_c, "wp": wp_c,
                        "bqk": bqk_c, "vbias": vbias_c})
    return in_maps


def assemble(results, b_proj):
    out = np.empty((B, S, D), np.float32)
    bp = np.asarray(b_proj, np.float32)
    for b in range(B):
        out[b] = results[2 * b]["out"] + results[2 * b + 1]["out"] + bp
    return out


def kernel(x, w_qkv, b_qkv, w_proj, b_proj):
    nc = get_nc()
    in_maps = make_in_maps(x, w_qkv, b_qkv, w_proj)
    res = bass_utils.run_bass_kernel_spmd(nc, in_maps, core_ids=list(range(8)),
                                          trace=False)
    return assemble(res.results, b_proj)



# revision 12
# speedup vs baseline: 29.8337x; 29.8337x over previous
"""Multi-head self-attention (B=4, S=2048, D=1024, H=16) on 8 TRN2 NeuronCores.

Sharding: batch x head-group. Core c handles batch b=c//2 and heads
[8*(c%2), 8*(c%2)+8). Each core computes QKV projection, attention and its
partial output projection; the host sums the two head-group partials per batch
and adds b_proj.

All matmul operands are bf16 (PSUM accumulation stays fp32), which halves
DMA/SBUF traffic at full PE rate. The attention phase is ACT(exp)-bound
(256 x [128,1024] Exp ~ 1.04us each), so stage 1 (QKV) is emitted pair-major
and dripped INTO the attention sweeps: after pair 0's Q/K/V are built, the
attention sweep for pair j runs while pair j+1's projection matmuls fill the
PE slack under the exp stream. Likewise the output projection drips into the
last sweep. Per-core dataflow:

  stage 1: Y^T = [Q^T; K^T] feature-tiles, V per pair (ones-columns make PV
           row 64 the softmax denominator)
  stage 2: S^T[k,q] = K_h Q_h^T via row-tiled head pairs (d=64 contraction,
           even head rows 0:64 / odd rows 64:128 run concurrently on HW)
  stage 3: C~^T = [V_h|1]^T P^T, P^T = exp(S^T/8) on ACT
           normalize: recip(sums) -> gpsimd partition all-reduce bcast -> mul
  stage 4: out = C^T-proj per token tile, dripped into sweep 3 + tail
"""
import numpy as np

import concourse.bacc as bacc
import concourse.tile as tile
from concourse import bass_isa, mybir
from concourse import bass_utils

P = 128
B, S, D = 4, 2048, 1024
H_TOT, HD = 16, 64
H = 8          # heads per core
NPAIR = 4      # head pairs per core
SCALE = HD ** -0.5
DCH = D // P   # 8 contraction chunks
NTT = S // P   # 16 token tiles
TCH = 256      # stage-1 token chunk
NCH = S // TCH
f32 = mybir.dt.float32
bf16 = mybir.dt.bfloat16
AF = mybir.ActivationFunctionType

_CACHED_NC = None


def build_nc(n_iter=1):
    """n_iter > 1 wraps the whole kernel body in a hardware loop executing
    it n_iter times back-to-back; a timing harness can then take the slope
    between two loop counts to get per-execution device time with the
    dispatch floor cancelled exactly. The graded kernel path uses n_iter=1."""
    import contextlib
    nc = bacc.Bacc(trn_type="TRN2", target_bir_lowering=False, debug=False)
    xt = nc.dram_tensor("xt", [D, S], bf16, kind="ExternalInput").ap()
    wqk = nc.dram_tensor("wqk", [D, 2 * H * HD], bf16, kind="ExternalInput").ap()
    wv = nc.dram_tensor("wv", [D, H * HD], bf16, kind="ExternalInput").ap()
    wp = nc.dram_tensor("wp", [H * HD, D], bf16, kind="ExternalInput").ap()
    bqk = nc.dram_tensor("bqk", [8, P], f32, kind="ExternalInput").ap()
    vbias = nc.dram_tensor("vbias", [P, NPAIR * 130], bf16, kind="ExternalInput").ap()
    out = nc.dram_tensor("out", [S, D], f32, kind="ExternalOutput").ap()
    # tiny passthrough pair so a timing harness can chain back-to-back
    # executions with a true data dependency (tock_i -> tick_{i+1})
    tick = nc.dram_tensor("tick", [1, P], f32, kind="ExternalInput").ap()
    tock = nc.dram_tensor("tock", [1, P], f32, kind="ExternalOutput").ap()

    with tile.TileContext(nc) as tc:
        with (
            tc.For_i(0, n_iter) if n_iter > 1 else contextlib.nullcontext(),
            tc.tile_pool(name="persist", bufs=1) as pp,
            tc.tile_pool(name="att", bufs=1) as att,
            tc.tile_pool(name="s4o", bufs=2) as s4o,
            tc.tile_pool(name="spt", bufs=2, space="PSUM") as sptp,
            tc.tile_pool(name="cps", bufs=2, space="PSUM") as cpsp,
            # one shared-tag PSUM pool for all dripped matmul groups
            # (stage-1 QK/V and the output projection): 2 rotating 2KB slots
            tc.tile_pool(name="dps", bufs=2, space="PSUM") as dpsp,
        ):
            # persistent SBUF tensors
            yt = [pp.tile([P, S], bf16, name=f"yt{f}") for f in range(8)]
            vp = pp.tile([P, NTT, NPAIR * 130], bf16, name="vp")
            xt_t = pp.tile([P, DCH, S], bf16, name="xt_t")
            wqk_f = [pp.tile([P, DCH, P], bf16, name=f"wqkf{f}") for f in range(8)]
            wv_t = pp.tile([P, DCH, H * HD], bf16, name="wv_t")
            wp_t = pp.tile([P, NPAIR, D], bf16, name="wp_t")
            bqk_t = pp.tile([P, 8], f32, name="bqk_t")
            vbias_t = pp.tile([P, NPAIR * 130], bf16, name="vbias_t")
            ct = [pp.tile([P, S], bf16, name=f"ct{j}") for j in range(NPAIR)]

            tick_t = pp.tile([1, P], f32, name="tick_t")
            nc.sync.dma_start(tick_t[:], tick[:])
            nc.sync.dma_start(tock[:], tick_t[:])

            # -------- input DMA (weights on ACT ring, xt on sync ring) ----
            xt4 = xt.rearrange("(c p) s -> p c s", p=P)
            nc.sync.dma_start(xt_t[:, :, 0:TCH], xt4[:, :, 0:TCH])
            wqk4 = wqk.rearrange("(c p) (f g) -> p c f g", p=P, f=8)
            nc.scalar.dma_start(wqk_f[0][:], wqk4[:, :, 0, :])
            nc.scalar.dma_start(wqk_f[4][:], wqk4[:, :, 4, :])
            nc.scalar.dma_start(bqk_t[:], bqk.rearrange("a p -> p a"))
            nc.scalar.dma_start(vbias_t[:], vbias[:])
            nc.scalar.dma_start(wv_t[:], wv.rearrange("(c p) f -> p c f", p=P))
            for t in range(1, NCH):
                nc.sync.dma_start(xt_t[:, :, t * TCH:(t + 1) * TCH],
                                  xt4[:, :, t * TCH:(t + 1) * TCH])
            for f in (1, 5, 2, 6, 3, 7):
                nc.scalar.dma_start(wqk_f[f][:], wqk4[:, :, f, :])
            nc.scalar.dma_start(wp_t[:], wp.rearrange("(c p) f -> p c f", p=P))
            for tt2 in range(NTT):
                nc.gpsimd.dma_start(vp[:, tt2, :], vbias[:])

            # -------- stage-1 step closures (drip quanta) -----------------
            def qk_step(f, t):
                """Q or K feature tile f, token chunk t: 8 matmuls + bias."""
                def g():
                    tsl = slice(t * TCH, (t + 1) * TCH)
                    ps = dpsp.tile([P, TCH], f32, name="dps")
                    for i in range(DCH):
                        nc.tensor.matmul(
                            ps[:], wqk_f[f][:, i, :], xt_t[:, i, tsl],
                            start=(i == 0), stop=(i == DCH - 1))
                    nc.vector.tensor_scalar(
                        out=yt[f][:, tsl], in0=ps[:],
                        scalar1=bqk_t[:, f:f + 1], scalar2=None,
                        op0=mybir.AluOpType.add)
                return g

            def v_step(j, tt):
                """V features for pair j over one 128-token tile."""
                def g():
                    ps = dpsp.tile([P, P], f32, name="dps")
                    for i in range(DCH):
                        nc.tensor.matmul(
                            ps[:], xt_t[:, i, tt * P:(tt + 1) * P],
                            wv_t[:, i, j * P:(j + 1) * P],
                            start=(i == 0), stop=(i == DCH - 1))
                    vpt = vp[:, tt, j * 130:(j + 1) * 130].rearrange(
                        "p (k c) -> p k c", k=2)
                    vb4 = vbias_t[:, j * 130:(j + 1) * 130].rearrange(
                        "p (k c) -> p k c", k=2)
                    nc.vector.tensor_tensor(
                        out=vpt[:, :, 0:HD],
                        in0=ps[:].rearrange("p (k c) -> p k c", k=2),
                        in1=vb4[:, :, 0:HD], op=mybir.AluOpType.add)
                return g

            def s1_pair(j):
                # interleave Q/K/V tiles; costs are PE-ns estimates
                steps = []
                for t in range(NCH):
                    steps.append((880, qk_step(j, t)))
                    steps.append((880, qk_step(NPAIR + j, t)))
                    steps.append((450, v_step(j, 2 * t)))
                    steps.append((450, v_step(j, 2 * t + 1)))
                return steps

            # pair 0 runs up front (attention j=0 needs it)
            for _, st in s1_pair(0):
                st()

            # -------- projection step closures ---------------------------
            def proj_steps(tt):
                tsl = slice(tt * P, (tt + 1) * P)
                steps = []
                state = {}

                def mk_mm(half, fc):
                    def g():
                        if fc == 0:
                            state[half] = dpsp.tile([P, 512], f32, name="dps")
                        nc.tensor.matmul(
                            state[half][:], ct[fc][:, tsl],
                            wp_t[:, fc, half * 512:(half + 1) * 512],
                            start=(fc == 0), stop=(fc == NPAIR - 1))
                        if fc == NPAIR - 1:
                            o_sb = s4o.tile([P, 512], f32, name="o_sb", bufs=4)
                            nc.vector.tensor_copy(o_sb[:], state[half][:])
                            nc.sync.dma_start(
                                out[tsl, half * 512:(half + 1) * 512], o_sb[:])
                    return g

                for half in range(2):
                    for fc in range(NPAIR):
                        steps.append(mk_mm(half, fc))
                return steps

            # -------- attention: j-outer sweeps with dripped work ---------
            # zeros rows 0:63 + per-norm recip row 64; partition all-reduce
            # (add) then replicates the recip row across all partitions
            zt = att.tile([65, 1024], f32, name="zt", bufs=1)
            nc.vector.memset(zt[0:HD, :], 0.0)

            def emit_norm(j, qa, cps_e, cps_o):
                nc.vector.reciprocal(zt[64:65, 0:512], cps_e[64:65, :])
                nc.vector.reciprocal(zt[64:65, 512:1024], cps_o[64:65, :])
                rbc = att.tile([65, 1024], f32, name="rbc", bufs=2)
                nc.gpsimd.partition_all_reduce(
                    rbc[:], zt[:], channels=65, reduce_op=bass_isa.ReduceOp.add)
                nc.vector.tensor_mul(ct[j][0:HD, qa], cps_e[0:HD, :],
                                     rbc[0:HD, 0:512])
                cttmp = att.tile([HD, 512], bf16, name="cttmp", bufs=2)
                nc.vector.tensor_mul(cttmp[:], cps_o[0:HD, :],
                                     rbc[0:HD, 512:1024])
                nc.sync.dma_start(ct[j][HD:P, qa], cttmp[:])

            work = []      # drip queue: (cost_ns, closure)
            credit = 0.0
            norm_pending = None
            for j in range(NPAIR):
                # pair j's stage-1 must be fully emitted before this sweep
                # reads yt/vp; drain any leftovers from the previous sweep
                for _, st in work:
                    st()
                work.clear()
                if j < NPAIR - 1:
                    work.extend(s1_pair(j + 1))
                qt, kt = yt[j], yt[NPAIR + j]
                for qc in range(4):
                    qa = slice(qc * 512, (qc + 1) * 512)
                    cps_e = cps_o = None
                    pv_pending = None
                    for kc in range(NTT):
                        ksl = slice(kc * P, (kc + 1) * P)
                        spt = sptp.tile([P, 1024], f32, name="spt")
                        nc.tensor.matmul(spt[:, 0:512], kt[0:HD, ksl],
                                         qt[0:HD, qa], start=True, stop=True)
                        nc.tensor.matmul(spt[:, 512:1024], kt[HD:P, ksl],
                                         qt[HD:P, qa], start=True, stop=True)
                        ppt = att.tile([P, 1024], bf16, name="ppt", bufs=4)
                        nc.scalar.activation(ppt[:], spt[:], AF.Exp,
                                             scale=SCALE)
                        if kc == 1 and norm_pending is not None:
                            # previous (j,qc) normalization, deferred past
                            # this iteration's first two S/exp to hide its
                            # recip -> all-reduce -> mul chain
                            pj, pqc = norm_pending[0], norm_pending[1]
                            emit_norm(pj, slice(pqc * 512, (pqc + 1) * 512),
                                      norm_pending[2], norm_pending[3])
                            norm_pending = None
                            if pj == NPAIR - 1:
                                # last pair's ct block just flushed: its
                                # output projection can now drip
                                for tt in range(pqc * 4, (pqc + 1) * 4):
                                    work.extend((230, st)
                                                for st in proj_steps(tt))
                        if pv_pending is not None:
                            if cps_e is None:
                                cps_e = cpsp.tile([65, 512], f32, name="cps")
                                cps_o = cpsp.tile([65, 512], f32, name="cps")
                            _emit_pv(nc, cps_e, cps_o, vp, pv_pending[0],
                                     pv_pending[1], j)
                        pv_pending = (kc, ppt)
                        # drip queued stage-1/proj work into the PE slack
                        # under the exp stream (~360ns per kc slot)
                        credit = min(credit + 360, 2600)
                        while work and credit >= work[0][0]:
                            cost, st = work.pop(0)
                            credit -= cost
                            st()
                    _emit_pv(nc, cps_e, cps_o, vp, pv_pending[0],
                             pv_pending[1], j)
                    norm_pending = (j, qc, cps_e, cps_o)
            emit_norm(norm_pending[0],
                      slice(norm_pending[1] * 512, (norm_pending[1] + 1) * 512),
                      norm_pending[2], norm_pending[3])
            for _, st in work:
                st()
            for tt in range(12, 16):
                for st in proj_steps(tt):
                    st()

    nc.finalize()
    return nc


def _emit_pv(nc, cps_e, cps_o, vp, kc, ppt, j):
    nc.tensor.matmul(cps_e[0:65, :], vp[:, kc, j * 130:j * 130 + 65],
                     ppt[:, 0:512], start=(kc == 0), stop=(kc == NTT - 1))
    nc.tensor.matmul(cps_o[0:65, :], vp[:, kc, j * 130 + 65:j * 130 + 130],
                     ppt[:, 512:1024], start=(kc == 0), stop=(kc == NTT - 1))


def get_nc():
    global _CACHED_NC
    if _CACHED_NC is None:
        _CACHED_NC = build_nc()
    return _CACHED_NC


def make_in_maps(x, w_qkv, b_qkv, w_proj):
    """Host-side sharding: one input dict per core."""
    nbf = mybir.dt.np(bf16)
    w = np.asarray(w_qkv, np.float32).reshape(D, 3, H_TOT, HD)
    bq3 = np.asarray(b_qkv, np.float32).reshape(3, H_TOT, HD)
    in_maps = []
    for c in range(8):
        b, hg = c // 2, c % 2
        hs = slice(hg * H, (hg + 1) * H)
        wqk_c = np.ascontiguousarray(
            np.concatenate([w[:, 0, hs, :].reshape(D, H * HD),
                            w[:, 1, hs, :].reshape(D, H * HD)],
                           axis=1)).astype(nbf)
        wv_c = np.ascontiguousarray(w[:, 2, hs, :].reshape(D, H * HD)).astype(nbf)
        wp_c = np.ascontiguousarray(
            np.asarray(w_proj, np.float32).reshape(H_TOT, HD, D)[hs]
            .reshape(H * HD, D)).astype(nbf)
        bqk_c = np.ascontiguousarray(
            np.concatenate([bq3[0, hs].reshape(H * HD),
                            bq3[1, hs].reshape(H * HD)]).reshape(8, P))
        bv = bq3[2, hs].reshape(H * HD)
        vbias_c = np.zeros((P, NPAIR * 130), np.float32)
        for j in range(NPAIR):
            vbias_c[:, j * 130:j * 130 + HD] = bv[(2 * j) * HD:(2 * j + 1) * HD]
            vbias_c[:, j * 130 + HD] = 1.0
            vbias_c[:, j * 130 + 65:j * 130 + 65 + HD] = \
                bv[(2 * j + 1) * HD:(2 * j + 2) * HD]
            vbias_c[:, j * 130 + 129] = 1.0
        xt_c = np.ascontiguousarray(np.asarray(x[b], np.float32).T).astype(nbf)
        in_maps.append({"xt": xt_c, "wqk": wqk_c, "wv": wv_c, "wp": wp_c,
                       "bqk": bqk_c, "vbias": vbias_c.astype(nbf),
                        "tick": np.zeros((1, P), np.float32)})
    return in_maps


def assemble(results, b_proj):
    out = np.empty((B, S, D), np.float32)
    bp = np.asarray(b_proj, np.float32)
    for b in range(B):
        out[b] = results[2 * b]["out"] + results[2 * b + 1]["out"] + bp
    return out


def kernel(x, w_qkv, b_qkv, w_proj, b_proj):
    nc = get_nc()
    in_maps = make_in_maps(x, w_qkv, b_qkv, w_proj)
    res = bass_utils.run_bass_kernel_spmd(nc, in_maps, core_ids=list(range(8)),
                                          trace=False)
    return assemble(res.results, b_proj)


# revision 13
# speedup vs baseline: 43.0764x; 1.4439x over previous
"""Multi-head self-attention (B=4, S=2048, D=1024, H=16) on 8 TRN2 NeuronCores.

Sharding: batch x head-group. Core c handles batch b=c//2 and heads
[8*(c%2), 8*(c%2)+8). Each core computes QKV projection, attention and its
partial output projection; the host sums the two head-group partials per batch
and adds b_proj.

All matmul operands are bf16 (PSUM accumulation stays fp32), which halves
DMA/SBUF traffic at full PE rate. The attention phase is ACT(exp)-bound
(256 x [128,1024] Exp ~ 1.04us each), so stage 1 (QKV) is emitted pair-major
and dripped INTO the attention sweeps: after pair 0's Q/K/V are built, the
attention sweep for pair j runs while pair j+1's projection matmuls fill the
PE slack under the exp stream. Likewise the output projection drips into the
last sweep. Per-core dataflow:

  stage 1: Y^T = [Q^T; K^T] feature-tiles, V per pair (ones-columns make PV
           row 64 the softmax denominator)
  stage 2: S^T[k,q] = K_h Q_h^T via row-tiled head pairs (d=64 contraction,
           even head rows 0:64 / odd rows 64:128 run concurrently on HW)
  stage 3: C~^T = [V_h|1]^T P^T, P^T = exp(S^T/8) on ACT
           normalize: recip(sums) -> gpsimd partition all-reduce bcast -> mul
  stage 4: out = C^T-proj per token tile, dripped into sweep 3 + tail
"""
import numpy as np

import concourse.bacc as bacc
import concourse.tile as tile
from concourse import bass_isa, mybir
from concourse import bass_utils

P = 128
B, S, D = 4, 2048, 1024
H_TOT, HD = 16, 64
H = 8          # heads per core
NPAIR = 4      # head pairs per core
SCALE = HD ** -0.5
DCH = D // P   # 8 contraction chunks
NTT = S // P   # 16 token tiles
TCH = 256      # stage-1 token chunk
NCH = S // TCH
f32 = mybir.dt.float32
bf16 = mybir.dt.bfloat16
AF = mybir.ActivationFunctionType

_CACHED_NC = None


def build_nc(n_iter=1):
    """n_iter > 1 wraps the whole kernel body in a hardware loop executing
    it n_iter times back-to-back; a timing harness can then take the slope
    between two loop counts to get per-execution device time with the
    dispatch floor cancelled exactly. The graded kernel path uses n_iter=1."""
    import contextlib
    nc = bacc.Bacc(trn_type="TRN2", target_bir_lowering=False, debug=False)
    xt = nc.dram_tensor("xt", [D, S], bf16, kind="ExternalInput").ap()
    wqk = nc.dram_tensor("wqk", [D, 2 * H * HD], bf16, kind="ExternalInput").ap()
    wv = nc.dram_tensor("wv", [D, H * HD], bf16, kind="ExternalInput").ap()
    wp = nc.dram_tensor("wp", [H * HD, D], bf16, kind="ExternalInput").ap()
    bqk = nc.dram_tensor("bqk", [8, P], f32, kind="ExternalInput").ap()
    vbias = nc.dram_tensor("vbias", [P, NPAIR * 130], bf16, kind="ExternalInput").ap()
    out = nc.dram_tensor("out", [S, D], f32, kind="ExternalOutput").ap()
    # tiny passthrough pair so a timing harness can chain back-to-back
    # executions with a true data dependency (tock_i -> tick_{i+1})
    tick = nc.dram_tensor("tick", [1, P], f32, kind="ExternalInput").ap()
    tock = nc.dram_tensor("tock", [1, P], f32, kind="ExternalOutput").ap()

    with tile.TileContext(nc) as tc:
        with (
            tc.For_i(0, n_iter, staggered_reset=True)
            if n_iter > 1 else contextlib.nullcontext(),
            tc.tile_pool(name="persist", bufs=1) as pp,
            tc.tile_pool(name="att", bufs=1) as att,
            tc.tile_pool(name="s4o", bufs=2) as s4o,
            tc.tile_pool(name="spt", bufs=2, space="PSUM") as sptp,
            tc.tile_pool(name="cps", bufs=2, space="PSUM") as cpsp,
            # one shared-tag PSUM pool for all dripped matmul groups
            # (stage-1 QK/V and the output projection): 2 rotating 2KB slots
            tc.tile_pool(name="dps", bufs=2, space="PSUM") as dpsp,
        ):
            # persistent SBUF tensors
            yt = [pp.tile([P, S], bf16, name=f"yt{f}") for f in range(8)]
            vp = pp.tile([P, NTT, NPAIR * 130], bf16, name="vp")
            xt_t = pp.tile([P, DCH, S], bf16, name="xt_t")
            wqk_f = [pp.tile([P, DCH, P], bf16, name=f"wqkf{f}") for f in range(8)]
            wv_t = pp.tile([P, DCH, H * HD], bf16, name="wv_t")
            wp_t = pp.tile([P, NPAIR, D], bf16, name="wp_t")
            bqk_t = pp.tile([P, 8], f32, name="bqk_t")
            vbias_t = pp.tile([P, NPAIR * 130], bf16, name="vbias_t")
            ct = [pp.tile([P, S], bf16, name=f"ct{j}") for j in range(NPAIR)]

            tick_t = pp.tile([1, P], f32, name="tick_t")
            nc.sync.dma_start(tick_t[:], tick[:])
            nc.sync.dma_start(tock[:], tick_t[:])

            # -------- input DMA (weights on ACT ring, xt on sync ring) ----
            xt4 = xt.rearrange("(c p) s -> p c s", p=P)
            nc.sync.dma_start(xt_t[:, :, 0:TCH], xt4[:, :, 0:TCH])
            wqk4 = wqk.rearrange("(c p) (f g) -> p c f g", p=P, f=8)
            nc.scalar.dma_start(wqk_f[0][:], wqk4[:, :, 0, :])
            nc.scalar.dma_start(wqk_f[4][:], wqk4[:, :, 4, :])
            nc.scalar.dma_start(bqk_t[:], bqk.rearrange("a p -> p a"))
            nc.scalar.dma_start(vbias_t[:], vbias[:])
            nc.scalar.dma_start(wv_t[:], wv.rearrange("(c p) f -> p c f", p=P))
            for t in range(1, NCH):
                nc.sync.dma_start(xt_t[:, :, t * TCH:(t + 1) * TCH],
                                  xt4[:, :, t * TCH:(t + 1) * TCH])
            for f in (1, 5, 2, 6, 3, 7):
                nc.scalar.dma_start(wqk_f[f][:], wqk4[:, :, f, :])
            nc.scalar.dma_start(wp_t[:], wp.rearrange("(c p) f -> p c f", p=P))
            for tt2 in range(NTT):
                nc.gpsimd.dma_start(vp[:, tt2, :], vbias[:])

            # -------- stage-1 step closures (drip quanta) -----------------
            def qk_step(f, t):
                """Q or K feature tile f, token chunk t: 8 matmuls + bias."""
                def g():
                    tsl = slice(t * TCH, (t + 1) * TCH)
                    ps = dpsp.tile([P, TCH], f32, name="dps")
                    for i in range(DCH):
                        nc.tensor.matmul(
                            ps[:], wqk_f[f][:, i, :], xt_t[:, i, tsl],
                            start=(i == 0), stop=(i == DCH - 1))
                    nc.vector.tensor_scalar(
                        out=yt[f][:, tsl], in0=ps[:],
                        scalar1=bqk_t[:, f:f + 1], scalar2=None,
                        op0=mybir.AluOpType.add)
                return g

            def v_step(j, tt):
                """V features for pair j over one 128-token tile."""
                def g():
                    ps = dpsp.tile([P, P], f32, name="dps")
                    for i in range(DCH):
                        nc.tensor.matmul(
                            ps[:], xt_t[:, i, tt * P:(tt + 1) * P],
                            wv_t[:, i, j * P:(j + 1) * P],
                            start=(i == 0), stop=(i == DCH - 1))
                    vpt = vp[:, tt, j * 130:(j + 1) * 130].rearrange(
                        "p (k c) -> p k c", k=2)
                    vb4 = vbias_t[:, j * 130:(j + 1) * 130].rearrange(
                        "p (k c) -> p k c", k=2)
                    nc.vector.tensor_tensor(
                        out=vpt[:, :, 0:HD],
                        in0=ps[:].rearrange("p (k c) -> p k c", k=2),
                        in1=vb4[:, :, 0:HD], op=mybir.AluOpType.add)
                return g

            def s1_pair(j):
                # interleave Q/K/V tiles; costs are PE-ns estimates
                steps = []
                for t in range(NCH):
                    steps.append((880, qk_step(j, t)))
                    steps.append((880, qk_step(NPAIR + j, t)))
                    steps.append((450, v_step(j, 2 * t)))
                    steps.append((450, v_step(j, 2 * t + 1)))
                return steps

            # pair 0 runs up front (attention j=0 needs it)
            for _, st in s1_pair(0):
                st()

            # -------- projection step closures ---------------------------
            def proj_steps(tt):
                tsl = slice(tt * P, (tt + 1) * P)
                steps = []
                state = {}

                def mk_mm(half, fc):
                    def g():
                        if fc == 0:
                            state[half] = dpsp.tile([P, 512], f32, name="dps")
                        nc.tensor.matmul(
                            state[half][:], ct[fc][:, tsl],
                            wp_t[:, fc, half * 512:(half + 1) * 512],
                            start=(fc == 0), stop=(fc == NPAIR - 1))
                        if fc == NPAIR - 1:
                            o_sb = s4o.tile([P, 512], f32, name="o_sb", bufs=4)
                            nc.vector.tensor_copy(o_sb[:], state[half][:])
                            nc.sync.dma_start(
                                out[tsl, half * 512:(half + 1) * 512], o_sb[:])
                    return g

                for half in range(2):
                    for fc in range(NPAIR):
                        steps.append(mk_mm(half, fc))
                return steps

            # -------- attention: j-outer sweeps with dripped work ---------
            # zeros rows 0:63 + per-norm recip row 64; partition all-reduce
            # (add) then replicates the recip row across all partitions
            zt = att.tile([65, 1024], f32, name="zt", bufs=1)
            nc.vector.memset(zt[0:HD, :], 0.0)

            def emit_norm(j, qa, cps_e, cps_o):
                nc.vector.reciprocal(zt[64:65, 0:512], cps_e[64:65, :])
                nc.vector.reciprocal(zt[64:65, 512:1024], cps_o[64:65, :])
                rbc = att.tile([65, 1024], f32, name="rbc", bufs=2)
                nc.gpsimd.partition_all_reduce(
                    rbc[:], zt[:], channels=65, reduce_op=bass_isa.ReduceOp.add)
                nc.vector.tensor_mul(ct[j][0:HD, qa], cps_e[0:HD, :],
                                     rbc[0:HD, 0:512])
                cttmp = att.tile([HD, 512], bf16, name="cttmp", bufs=2)
                nc.vector.tensor_mul(cttmp[:], cps_o[0:HD, :],
                                     rbc[0:HD, 512:1024])
                nc.sync.dma_start(ct[j][HD:P, qa], cttmp[:])

            work = []      # drip queue: (cost_ns, closure)
            credit = 0.0
            norm_pending = None
            for j in range(NPAIR):
                # pair j's stage-1 must be fully emitted before this sweep
                # reads yt/vp; drain any leftovers from the previous sweep
                for _, st in work:
                    st()
                work.clear()
                if j < NPAIR - 1:
                    work.extend(s1_pair(j + 1))
                qt, kt = yt[j], yt[NPAIR + j]
                for qc in range(4):
                    qa = slice(qc * 512, (qc + 1) * 512)
                    cps_e = cps_o = None
                    pv_pending = None
                    for kc in range(NTT):
                        ksl = slice(kc * P, (kc + 1) * P)
                        spt = sptp.tile([P, 1024], f32, name="spt")
                        nc.tensor.matmul(spt[:, 0:512], kt[0:HD, ksl],
                                         qt[0:HD, qa], start=True, stop=True)
                        nc.tensor.matmul(spt[:, 512:1024], kt[HD:P, ksl],
                                         qt[HD:P, qa], start=True, stop=True)
                        ppt = att.tile([P, 1024], bf16, name="ppt", bufs=4)
                        nc.scalar.activation(ppt[:], spt[:], AF.Exp,
                                             scale=SCALE)
                        if kc == 1 and norm_pending is not None:
                            # previous (j,qc) normalization, deferred past
                            # this iteration's first two S/exp to hide its
                            # recip -> all-reduce -> mul chain
                            pj, pqc = norm_pending[0], norm_pending[1]
                            emit_norm(pj, slice(pqc * 512, (pqc + 1) * 512),
                                      norm_pending[2], norm_pending[3])
                            norm_pending = None
                            if pj == NPAIR - 1:
                                # last pair's ct block just flushed: its
                                # output projection can now drip
                                for tt in range(pqc * 4, (pqc + 1) * 4):
                                    work.extend((230, st)
                                                for st in proj_steps(tt))
                        if pv_pending is not None:
                            if cps_e is None:
                                cps_e = cpsp.tile([65, 512], f32, name="cps")
                                cps_o = cpsp.tile([65, 512], f32, name="cps")
                            _emit_pv(nc, cps_e, cps_o, vp, pv_pending[0],
                                     pv_pending[1], j)
                        pv_pending = (kc, ppt)
                        # drip queued stage-1/proj work into the PE slack
                        # under the exp stream (~360ns per kc slot)
                        credit = min(credit + 360, 2600)
                        while work and credit >= work[0][0]:
                            cost, st = work.pop(0)
                            credit -= cost
                            st()
                    _emit_pv(nc, cps_e, cps_o, vp, pv_pending[0],
                             pv_pending[1], j)
                    norm_pending = (j, qc, cps_e, cps_o)
            emit_norm(norm_pending[0],
                      slice(norm_pending[1] * 512, (norm_pending[1] + 1) * 512),
                      norm_pending[2], norm_pending[3])
            for _, st in work:
                st()
            for tt in range(12, 16):
                for st in proj_steps(tt):
                    st()

    nc.finalize()
    return nc


def _emit_pv(nc, cps_e, cps_o, vp, kc, ppt, j):
    nc.tensor.matmul(cps_e[0:65, :], vp[:, kc, j * 130:j * 130 + 65],
                     ppt[:, 0:512], start=(kc == 0), stop=(kc == NTT - 1))
    nc.tensor.matmul(cps_o[0:65, :], vp[:, kc, j * 130 + 65:j * 130 + 130],
                     ppt[:, 512:1024], start=(kc == 0), stop=(kc == NTT - 1))


def get_nc():
    global _CACHED_NC
    if _CACHED_NC is None:
        _CACHED_NC = build_nc()
    return _CACHED_NC


def make_in_maps(x, w_qkv, b_qkv, w_proj):
    """Host-side sharding: one input dict per core."""
    nbf = mybir.dt.np(bf16)
    w = np.asarray(w_qkv, np.float32).reshape(D, 3, H_TOT, HD)
    bq3 = np.asarray(b_qkv, np.float32).reshape(3, H_TOT, HD)
    in_maps = []
    for c in range(8):
        b, hg = c // 2, c % 2
        hs = slice(hg * H, (hg + 1) * H)
        wqk_c = np.ascontiguousarray(
            np.concatenate([w[:, 0, hs, :].reshape(D, H * HD),
                            w[:, 1, hs, :].reshape(D, H * HD)],
                           axis=1)).astype(nbf)
        wv_c = np.ascontiguousarray(w[:, 2, hs, :].reshape(D, H * HD)).astype(nbf)
        wp_c = np.ascontiguousarray(
            np.asarray(w_proj, np.float32).reshape(H_TOT, HD, D)[hs]
            .reshape(H * HD, D)).astype(nbf)
        bqk_c = np.ascontiguousarray(
            np.concatenate([bq3[0, hs].reshape(H * HD),
                            bq3[1, hs].reshape(H * HD)]).reshape(8, P))
        bv = bq3[2, hs].reshape(H * HD)
        vbias_c = np.zeros((P, NPAIR * 130), np.float32)
        for j in range(NPAIR):
            vbias_c[:, j * 130:j * 130 + HD] = bv[(2 * j) * HD:(2 * j + 1) * HD]
            vbias_c[:, j * 130 + HD] = 1.0
            vbias_c[:, j * 130 + 65:j * 130 + 65 + HD] = \
                bv[(2 * j + 1) * HD:(2 * j + 2) * HD]
            vbias_c[:, j * 130 + 129] = 1.0
        xt_c = np.ascontiguousarray(np.asarray(x[b], np.float32).T).astype(nbf)
        in_maps.append({"xt": xt_c, "wqk": wqk_c, "wv": wv_c, "wp": wp_c,
                       "bqk": bqk_c, "vbias": vbias_c.astype(nbf),
                        "tick": np.zeros((1, P), np.float32)})
    return in_maps


def assemble(results, b_proj):
    out = np.empty((B, S, D), np.float32)
    bp = np.asarray(b_proj, np.float32)
    for b in range(B):
        out[b] = results[2 * b]["out"] + results[2 * b + 1]["out"] + bp
    return out


def kernel(x, w_qkv, b_qkv, w_proj, b_proj):
    nc = get_nc()
    in_maps = make_in_maps(x, w_qkv, b_qkv, w_proj)
    res = bass_utils.run_bass_kernel_spmd(nc, in_maps, core_ids=list(range(8)),
                                          trace=False)
    return assemble(res.results, b_proj)


# revision 18
# speedup vs baseline: 51.3242x; 1.1915x over previous
"""Multi-head self-attention (B=4, S=2048, D=1024, H=16) on 8 TRN2 NeuronCores.

Sharding: batch x head-group. Core c handles batch b=c//2 and heads
[8*(c%2), 8*(c%2)+8). Each core computes QKV projection, attention and its
partial output projection; the host sums the two head-group partials per batch
and adds b_proj.

All matmul operands are bf16 (PSUM accumulation stays fp32), which halves
DMA/SBUF traffic at full PE rate. The attention phase is ACT(exp)-bound
(256 x [128,1024] Exp ~ 1.04us each), so stage 1 (QKV) is emitted pair-major
and dripped INTO the attention sweeps: after pair 0's Q/K/V are built, the
attention sweep for pair j runs while pair j+1's projection matmuls fill the
PE slack under the exp stream. Likewise the output projection drips into the
last sweep. Per-core dataflow:

  stage 1: Y^T = [Q^T; K^T] feature-tiles, V per pair (ones-columns make PV
           row 64 the softmax denominator)
  stage 2: S^T[k,q] = K_h Q_h^T via row-tiled head pairs (d=64 contraction,
           even head rows 0:64 / odd rows 64:128 run concurrently on HW)
  stage 3: C~^T = [V_h|1]^T P^T, P^T = exp(S^T/8) on ACT
           normalize: recip(sums) -> gpsimd partition all-reduce bcast -> mul
  stage 4: out = C^T-proj per token tile, dripped into sweep 3 + tail
"""
import numpy as np

import concourse.bacc as bacc
import concourse.tile as tile
from concourse import bass_isa, mybir
from concourse import bass_utils

P = 128
B, S, D = 4, 2048, 1024
H_TOT, HD = 16, 64
H = 8          # heads per core
NPAIR = 4      # head pairs per core
SCALE = HD ** -0.5
DCH = D // P   # 8 contraction chunks
NTT = S // P   # 16 token tiles
TCH = 256      # stage-1 token chunk
NCH = S // TCH
f32 = mybir.dt.float32
bf16 = mybir.dt.bfloat16
AF = mybir.ActivationFunctionType

_CACHED_NC = None


def build_nc(n_iter=1):
    """n_iter > 1 wraps the whole kernel body in a hardware loop executing
    it n_iter times back-to-back; a timing harness can then take the slope
    between two loop counts to get per-execution device time with the
    dispatch floor cancelled exactly. The graded kernel path uses n_iter=1."""
    import contextlib
    nc = bacc.Bacc(trn_type="TRN2", target_bir_lowering=False, debug=False)
    xt = nc.dram_tensor("xt", [D, S], bf16, kind="ExternalInput").ap()
    wqk = nc.dram_tensor("wqk", [D, 2 * H * HD], bf16, kind="ExternalInput").ap()
    wv = nc.dram_tensor("wv", [D, H * HD], bf16, kind="ExternalInput").ap()
    wp = nc.dram_tensor("wp", [H * HD, D], bf16, kind="ExternalInput").ap()
    bqk = nc.dram_tensor("bqk", [8, P], f32, kind="ExternalInput").ap()
    vbias = nc.dram_tensor("vbias", [P, NPAIR * 130], bf16, kind="ExternalInput").ap()
    out = nc.dram_tensor("out", [S, D], f32, kind="ExternalOutput").ap()
    # tiny passthrough pair so a timing harness can chain back-to-back
    # executions with a true data dependency (tock_i -> tick_{i+1})
    tick = nc.dram_tensor("tick", [1, P], f32, kind="ExternalInput").ap()
    tock = nc.dram_tensor("tock", [1, P], f32, kind="ExternalOutput").ap()

    with tile.TileContext(nc) as tc:
        with (
            tc.For_i(0, n_iter, staggered_reset=True)
            if n_iter > 1 else contextlib.nullcontext(),
            tc.tile_pool(name="persist", bufs=1) as pp,
            tc.tile_pool(name="att", bufs=1) as att,
            tc.tile_pool(name="s4o", bufs=2) as s4o,
            tc.tile_pool(name="spt", bufs=2, space="PSUM") as sptp,
            tc.tile_pool(name="cps", bufs=2, space="PSUM") as cpsp,
            # one shared-tag PSUM pool for all dripped matmul groups
            # (stage-1 QK/V and the output projection): 2 rotating 2KB slots
            tc.tile_pool(name="dps", bufs=2, space="PSUM") as dpsp,
        ):
            # persistent SBUF tensors
            yt = [pp.tile([P, S], bf16, name=f"yt{f}") for f in range(8)]
            vp = pp.tile([P, NTT, NPAIR * 130], bf16, name="vp")
            xt_t = pp.tile([P, DCH, S], bf16, name="xt_t")
            wqk_f = [pp.tile([P, DCH, P], bf16, name=f"wqkf{f}") for f in range(8)]
            wv_t = pp.tile([P, DCH, H * HD], bf16, name="wv_t")
            wp_t = pp.tile([P, NPAIR, D], bf16, name="wp_t")
            bqk_t = pp.tile([P, 8], f32, name="bqk_t")
            vbias_t = pp.tile([P, NPAIR * 130], bf16, name="vbias_t")
            ct = [pp.tile([P, S], bf16, name=f"ct{j}") for j in range(NPAIR)]

            tick_t = pp.tile([1, P], f32, name="tick_t")
            nc.sync.dma_start(tick_t[:], tick[:])
            nc.sync.dma_start(tock[:], tick_t[:])

            # -------- input DMA (weights on ACT ring, xt on sync ring) ----
            xt4 = xt.rearrange("(c p) s -> p c s", p=P)
            nc.sync.dma_start(xt_t[:, :, 0:TCH], xt4[:, :, 0:TCH])
            wqk4 = wqk.rearrange("(c p) (f g) -> p c f g", p=P, f=8)
            nc.scalar.dma_start(wqk_f[0][:], wqk4[:, :, 0, :])
            nc.scalar.dma_start(wqk_f[4][:], wqk4[:, :, 4, :])
            nc.scalar.dma_start(bqk_t[:], bqk.rearrange("a p -> p a"))
            nc.scalar.dma_start(vbias_t[:], vbias[:])
            nc.scalar.dma_start(wv_t[:], wv.rearrange("(c p) f -> p c f", p=P))
            for t in range(1, NCH):
                nc.sync.dma_start(xt_t[:, :, t * TCH:(t + 1) * TCH],
                                  xt4[:, :, t * TCH:(t + 1) * TCH])
            for f in (1, 5, 2, 6, 3, 7):
                nc.scalar.dma_start(wqk_f[f][:], wqk4[:, :, f, :])
            nc.scalar.dma_start(wp_t[:], wp.rearrange("(c p) f -> p c f", p=P))
            for tt2 in range(NTT):
                nc.gpsimd.dma_start(vp[:, tt2, :], vbias[:])

            # -------- stage-1 step closures (drip quanta) -----------------
            def qk_step(f, t):
                """Q or K feature tile f, token chunk t: 8 matmuls + bias."""
                def g():
                    tsl = slice(t * TCH, (t + 1) * TCH)
                    ps = dpsp.tile([P, TCH], f32, name="dps")
                    for i in range(DCH):
                        nc.tensor.matmul(
                            ps[:], wqk_f[f][:, i, :], xt_t[:, i, tsl],
                            start=(i == 0), stop=(i == DCH - 1))
                    nc.vector.tensor_scalar(
                        out=yt[f][:, tsl], in0=ps[:],
                        scalar1=bqk_t[:, f:f + 1], scalar2=None,
                        op0=mybir.AluOpType.add)
                return g

            def v_step(j, tt):
                """V features for pair j over one 128-token tile."""
                def g():
                    ps = dpsp.tile([P, P], f32, name="dps")
                    for i in range(DCH):
                        nc.tensor.matmul(
                            ps[:], xt_t[:, i, tt * P:(tt + 1) * P],
                            wv_t[:, i, j * P:(j + 1) * P],
                            start=(i == 0), stop=(i == DCH - 1))
                    vpt = vp[:, tt, j * 130:(j + 1) * 130].rearrange(
                        "p (k c) -> p k c", k=2)
                    vb4 = vbias_t[:, j * 130:(j + 1) * 130].rearrange(
                        "p (k c) -> p k c", k=2)
                    nc.vector.tensor_tensor(
                        out=vpt[:, :, 0:HD],
                        in0=ps[:].rearrange("p (k c) -> p k c", k=2),
                        in1=vb4[:, :, 0:HD], op=mybir.AluOpType.add)
                return g

            def s1_pair(j):
                # interleave Q/K/V tiles; costs are PE-ns estimates
                steps = []
                for t in range(NCH):
                    steps.append((880, qk_step(j, t)))
                    steps.append((880, qk_step(NPAIR + j, t)))
                    steps.append((450, v_step(j, 2 * t)))
                    steps.append((450, v_step(j, 2 * t + 1)))
                return steps

            # pair 0 runs up front (attention j=0 needs it)
            for _, st in s1_pair(0):
                st()

            # -------- projection step closures ---------------------------
            def proj_steps(tt):
                tsl = slice(tt * P, (tt + 1) * P)
                steps = []
                state = {}

                def mk_mm(half, fc):
                    def g():
                        if fc == 0:
                            state[half] = dpsp.tile([P, 512], f32, name="dps")
                        nc.tensor.matmul(
                            state[half][:], ct[fc][:, tsl],
                            wp_t[:, fc, half * 512:(half + 1) * 512],
                            start=(fc == 0), stop=(fc == NPAIR - 1))
                        if fc == NPAIR - 1:
                            o_sb = s4o.tile([P, 512], f32, name="o_sb", bufs=4)
                            nc.vector.tensor_copy(o_sb[:], state[half][:])
                            nc.sync.dma_start(
                                out[tsl, half * 512:(half + 1) * 512], o_sb[:])
                    return g

                for half in range(2):
                    for fc in range(NPAIR):
                        steps.append(mk_mm(half, fc))
                return steps

            # -------- attention: j-outer sweeps with dripped work ---------
            # zeros rows 0:63 + per-norm recip row 64; partition all-reduce
            # (add) then replicates the recip row across all partitions
            zt = att.tile([65, 1024], f32, name="zt", bufs=1)
            nc.vector.memset(zt[0:HD, :], 0.0)

            def emit_norm(j, qa, csb):
                nc.vector.reciprocal(zt[64:65, :], csb[64:65, :])
                rbc = att.tile([65, 1024], f32, name="rbc", bufs=2)
                nc.gpsimd.partition_all_reduce(
                    rbc[:], zt[:], channels=65, reduce_op=bass_isa.ReduceOp.add)
                nc.vector.tensor_mul(ct[j][0:HD, qa], csb[0:HD, 0:512],
                                     rbc[0:HD, 0:512])
                cttmp = att.tile([HD, 512], bf16, name="cttmp", bufs=2)
                nc.vector.tensor_mul(cttmp[:], csb[0:HD, 512:1024],
                                     rbc[0:HD, 512:1024])
                nc.sync.dma_start(ct[j][HD:P, qa], cttmp[:])

            work = []      # drip queue: (cost_ns, closure)
            credit = 0.0
            norm_pending = None
            for j in range(NPAIR):
                # pair j's stage-1 must be fully emitted before this sweep
                # reads yt/vp; drain any leftovers from the previous sweep
                for _, st in work:
                    st()
                work.clear()
                if j < NPAIR - 1:
                    work.extend(s1_pair(j + 1))
                qt, kt = yt[j], yt[NPAIR + j]
                for qc in range(4):
                    qa = slice(qc * 512, (qc + 1) * 512)
                    cps_e = cps_o = None
                    pv_pending = None
                    for kc in range(NTT):
                        ksl = slice(kc * P, (kc + 1) * P)
                        spt = sptp.tile([P, 1024], f32, name="spt")
                        nc.tensor.matmul(spt[:, 0:512], kt[0:HD, ksl],
                                         qt[0:HD, qa], start=True, stop=True)
                        nc.tensor.matmul(spt[:, 512:1024], kt[HD:P, ksl],
                                         qt[HD:P, qa], start=True, stop=True)
                        ppt = att.tile([P, 1024], bf16, name="ppt", bufs=6)
                        nc.scalar.activation(ppt[:], spt[:], AF.Exp,
                                             scale=SCALE)
                        if kc == 1 and norm_pending is not None:
                            # previous (j,qc) normalization, deferred past
                            # this iteration's first two S/exp to hide its
                            # recip -> all-reduce -> mul chain
                            pj, pqc = norm_pending[0], norm_pending[1]
                            emit_norm(pj, slice(pqc * 512, (pqc + 1) * 512),
                                      norm_pending[2])
                            norm_pending = None
                            if pj == NPAIR - 1:
                                # last pair's ct block just flushed: its
                                # output projection can now drip
                                for tt in range(pqc * 4, (pqc + 1) * 4):
                                    work.extend((230, st)
                                                for st in proj_steps(tt))
                        if pv_pending is not None:
                            if cps_e is None:
                                cps_e = cpsp.tile([65, 512], f32, name="cps")
                                cps_o = cpsp.tile([65, 512], f32, name="cps")
                            _emit_pv(nc, cps_e, cps_o, vp, pv_pending[0],
                                     pv_pending[1], j)
                        pv_pending = (kc, ppt)
                        # drip queued stage-1/proj work into the PE slack
                        # under the exp stream (~450ns per kc slot)
                        credit = min(credit + 450, 2600)
                        while work and credit >= work[0][0]:
                            cost, st = work.pop(0)
                            credit -= cost
                            st()
                    _emit_pv(nc, cps_e, cps_o, vp, pv_pending[0],
                             pv_pending[1], j)
                    # evacuate the PV accumulators to SBUF right away so the
                    # next generation's PV is not blocked on the (slow)
                    # normalization chain still reading these PSUM banks
                    csb = att.tile([65, 1024], f32, name="csb", bufs=2)
                    nc.vector.tensor_copy(csb[:, 0:512], cps_e[:])
                    nc.vector.tensor_copy(csb[:, 512:1024], cps_o[:])
                    norm_pending = (j, qc, csb)
            emit_norm(norm_pending[0],
                      slice(norm_pending[1] * 512, (norm_pending[1] + 1) * 512),
                      norm_pending[2])
            for _, st in work:
                st()
            for tt in range(12, 16):
                for st in proj_steps(tt):
                    st()

    nc.finalize()
    return nc


def _emit_pv(nc, cps_e, cps_o, vp, kc, ppt, j):
    nc.tensor.matmul(cps_e[0:65, :], vp[:, kc, j * 130:j * 130 + 65],
                     ppt[:, 0:512], start=(kc == 0), stop=(kc == NTT - 1))
    nc.tensor.matmul(cps_o[0:65, :], vp[:, kc, j * 130 + 65:j * 130 + 130],
                     ppt[:, 512:1024], start=(kc == 0), stop=(kc == NTT - 1))


def get_nc():
    global _CACHED_NC
    if _CACHED_NC is None:
        _CACHED_NC = build_nc()
    return _CACHED_NC


def make_in_maps(x, w_qkv, b_qkv, w_proj):
    """Host-side sharding: one input dict per core."""
    nbf = mybir.dt.np(bf16)
    w = np.asarray(w_qkv, np.float32).reshape(D, 3, H_TOT, HD)
    bq3 = np.asarray(b_qkv, np.float32).reshape(3, H_TOT, HD)
    in_maps = []
    for c in range(8):
        b, hg = c // 2, c % 2
        hs = slice(hg * H, (hg + 1) * H)
        wqk_c = np.ascontiguousarray(
            np.concatenate([w[:, 0, hs, :].reshape(D, H * HD),
                            w[:, 1, hs, :].reshape(D, H * HD)],
                           axis=1)).astype(nbf)
        wv_c = np.ascontiguousarray(w[:, 2, hs, :].reshape(D, H * HD)).astype(nbf)
        wp_c = np.ascontiguousarray(
            np.asarray(w_proj, np.float32).reshape(H_TOT, HD, D)[hs]
            .reshape(H * HD, D)).astype(nbf)
        bqk_c = np.ascontiguousarray(
            np.concatenate([bq3[0, hs].reshape(H * HD),
                            bq3[1, hs].reshape(H * HD)]).reshape(8, P))
        bv = bq3[2, hs].reshape(H * HD)
        vbias_c = np.zeros((P, NPAIR * 130), np.float32)
        for j in range(NPAIR):
            vbias_c[:, j * 130:j * 130 + HD] = bv[(2 * j) * HD:(2 * j + 1) * HD]
            vbias_c[:, j * 130 + HD] = 1.0
            vbias_c[:, j * 130 + 65:j * 130 + 65 + HD] = \
                bv[(2 * j + 1) * HD:(2 * j + 2) * HD]
            vbias_c[:, j * 130 + 129] = 1.0
        xt_c = np.ascontiguousarray(np.asarray(x[b], np.float32).T).astype(nbf)
        in_maps.append({"xt": xt_c, "wqk": wqk_c, "wv": wv_c, "wp": wp_c,
                       "bqk": bqk_c, "vbias": vbias_c.astype(nbf),
                        "tick": np.zeros((1, P), np.float32)})
    return in_maps


def assemble(results, b_proj):
    out = np.empty((B, S, D), np.float32)
    bp = np.asarray(b_proj, np.float32)
    for b in range(B):
        out[b] = results[2 * b]["out"] + results[2 * b + 1]["out"] + bp
    return out


def kernel(x, w_qkv, b_qkv, w_proj, b_proj):
    nc = get_nc()
    in_maps = make_in_maps(x, w_qkv, b_qkv, w_proj)
    res = bass_utils.run_bass_kernel_spmd(nc, in_maps, core_ids=list(range(8)),
                                          trace=False)
    return assemble(res.results, b_proj)
